# revision 13
# baseline (speedup 1.0000x reference)
# Trainium2 Bass kernel for a 2-layer GPT-NeoX-style dense transformer.
#
# v2: S-chunk software pipeline (4 chunks of 512 tokens).
#   - Megatron TP over 8 cores: qkv/fc1 column-sharded, attn_out/fc2
#     row-sharded; one fused bf16 AllReduce per (layer, chunk) that carries
#     attn+mlp partial sums; residual stream replicated in DRAM as bf16.
#   - Per-chunk LN (stats per token need only that chunk), so collectives
#     overlap neighboring chunks' GEMMs instead of serializing layers.
#   - Attention PV is computed output-transposed (ctxT = sum_kb V^T e), which
#     kills the ctx transposes and makes PV matmuls N=512.
#   - Softmax uses exp-without-max (scores provably small); denominator from
#     a ones-vector matmul; normalization folded into the ctxT PSUM evac.
#   - Emission interleaves F(n)=LN+QKV+V+fc1 and B(n)=attn+out+AR as
#     F0 F1 B0 F2 B1 F3 B2 B3 so every engine queue keeps one chunk of
#     lookahead while PSUM stays within 8 banks.
import math
from contextlib import ExitStack

import numpy as np
import ml_dtypes

import concourse.bass as bass
import concourse.bass_isa as bass_isa
import concourse.bacc as bacc
import concourse.tile as tile
import concourse.mybir as mybir
from concourse.bass_utils import run_bass_kernel_spmd
from concourse.masks import make_identity

F32 = mybir.dt.float32
BF16 = mybir.dt.bfloat16
I32 = mybir.dt.int32
BFNP = ml_dtypes.bfloat16
AF = mybir.ActivationFunctionType
OP = mybir.AluOpType

NC = 8
EPS = 1e-5
BASE = 10000.0

REAL_CFG = dict(S=2048, H=2048, NH=16, FF=8192, V=32000, L=2)


def derive(cfg):
    d = dict(cfg)
    d["HD"] = 128
    d["ROT"] = 32
    d["NHL"] = cfg["NH"] // NC          # heads per core (2)
    d["NBLK"] = 2 * d["NHL"]            # q/k 128-row blocks per core (4)
    d["QKR"] = d["NBLK"] * 128          # q+k rows per core (512)
    d["VCOL"] = d["NHL"] * 128          # v cols per core (256)
    d["FFL"] = cfg["FF"] // NC          # 1024
    d["FMT"] = d["FFL"] // 128          # 8
    d["VL"] = cfg["V"] // NC            # 4000
    d["EMB"] = cfg["H"] // NC           # 256
    d["EB"] = d["EMB"] // 128           # 2
    d["HT"] = cfg["H"] // 128           # 16 contraction tiles over H
    d["ST"] = cfg["S"] // 128           # 16
    d["CH"] = 512                       # S-chunk width
    d["NCH"] = cfg["S"] // 512          # 4 chunks
    d["MB"] = 512 // 128                # 4 m-blocks per chunk
    d["VN"] = 500
    d["VNT"] = d["VL"] // 500           # 8
    return d


# ---------------------------------------------------------------- program ---


def build_program(cfg):
    c = derive(cfg)
    S, H, L = c["S"], c["H"], c["L"]
    HT, CH, NCH, MB = c["HT"], c["CH"], c["NCH"], c["MB"]
    NHL, VCOL, FMT, FFL = c["NHL"], c["VCOL"], c["FMT"], c["FFL"]

    nc = bacc.Bacc("TRN2", target_bir_lowering=False, debug=False, num_devices=NC)

    # inputs
    tokens_t = nc.dram_tensor("tokens_t", [128, c["ST"]], I32, kind="ExternalInput").ap()
    embed_hs = nc.dram_tensor("embed_hs", [cfg["V"], c["EMB"]], F32, kind="ExternalInput").ap()
    cos32 = nc.dram_tensor("cos32", [32, S], BF16, kind="ExternalInput").ap()
    sin32 = nc.dram_tensor("sin32", [32, S], BF16, kind="ExternalInput").ap()
    tri = nc.dram_tensor("tri", [128, 128], BF16, kind="ExternalInput").ap()
    w_qkT = nc.dram_tensor("w_qkT", [L, H, c["QKR"]], BF16, kind="ExternalInput").ap()
    w_vT = nc.dram_tensor("w_vT", [L, H, VCOL], BF16, kind="ExternalInput").ap()
    w_oT = nc.dram_tensor("w_oT", [L, VCOL, H], BF16, kind="ExternalInput").ap()
    w_f1T = nc.dram_tensor("w_f1T", [L, H, FFL], BF16, kind="ExternalInput").ap()
    w_f2T = nc.dram_tensor("w_f2T", [L, FFL, H], BF16, kind="ExternalInput").ap()
    w_lgT = nc.dram_tensor("w_lgT", [H, c["VL"]], BF16, kind="ExternalInput").ap()
    b_qk = nc.dram_tensor("b_qk", [L, 128, c["NBLK"]], F32, kind="ExternalInput").ap()
    b_f1 = nc.dram_tensor("b_f1", [L, 128, FMT], F32, kind="ExternalInput").ap()
    b_out = nc.dram_tensor("b_out", [L, 128, HT], F32, kind="ExternalInput").ap()

    logits = nc.dram_tensor("logits", [S, c["VL"]], F32, kind="ExternalOutput").ap()

    rg = [list(range(NC))]

    with tile.TileContext(nc) as tc, ExitStack() as top:
        cp = top.enter_context(tc.tile_pool(name="const", bufs=1))
        gp = top.enter_context(tc.tile_pool(name="glob", bufs=1))
        pp = top.enter_context(tc.tile_pool(name="ps", bufs=1, space="PSUM"))
        dp = top.enter_context(tc.tile_pool(name="dram", bufs=1, space="DRAM"))

        # warm-up collective: absorbs the one-time CC entry barrier while
        # the embedding gathers run, so the first real AllGather starts hot
        warm_in = dp.tile([8, 1], BF16, name="warm_in")
        warm_out = dp.tile([64, 1], BF16, name="warm_out", addr_space="Shared")
        nc.gpsimd.collective_compute(
            "AllGather", OP.bypass, replica_groups=rg,
            ins=[warm_in[:]], outs=[warm_out[:]])

        # ---------------- constants
        ident_f = cp.tile([128, 128], F32, name="ident_f")
        make_identity(nc, ident_f[:])
        ones_b = cp.tile([128, 1], BF16, name="ones_b")
        nc.gpsimd.memset(ones_b[:], 1.0)
        zero_p = cp.tile([128, 1], F32, name="zero_p")
        nc.gpsimd.memset(zero_p[:], 0.0)
        nc.const_aps.aps[(F32, 0.0)] = zero_p[:]
        eps_p = cp.tile([128, 1], F32, name="eps_p")
        nc.gpsimd.memset(eps_p[:], EPS)
        nc.const_aps.aps[(F32, EPS)] = eps_p[:]
        tok_sb = cp.tile([128, c["ST"]], I32, name="tok_sb")
        nc.sync.dma_start(tok_sb[:], tokens_t[:])
        tri_sb = cp.tile([128, 128], BF16, name="tri_sb")
        nc.sync.dma_start(tri_sb[:], tri[:])
        bqk_sb = [cp.tile([128, c["NBLK"]], F32, name=f"bqk{l}") for l in range(L)]
        bf1_sb = [cp.tile([128, FMT], F32, name=f"bf1{l}") for l in range(L)]
        bout_sb = [cp.tile([128, HT], F32, name=f"bout{l}") for l in range(L)]
        for l in range(L):
            nc.sync.dma_start(bqk_sb[l][:], b_qk[l])
            nc.sync.dma_start(bf1_sb[l][:], b_f1[l])
            nc.sync.dma_start(bout_sb[l][:], b_out[l])

        # ---------------- DRAM intermediates (per chunk)
        h0s_c = [dp.tile([c["EMB"], CH], BF16, name=f"h0s_{n}") for n in range(NCH)]
        h0T_c = [dp.tile([H, CH], BF16, name=f"h0T_{n}", addr_space="Shared")
                 for n in range(NCH)]
        hb1_c = [dp.tile([H, CH], BF16, name=f"hb1_{n}") for n in range(NCH)]
        hb2_c = [dp.tile([H, CH], BF16, name=f"hb2_{n}") for n in range(NCH)]
        part = [[dp.tile([H, CH], BF16, name=f"part_{l}_{n}") for n in range(NCH)]
                for l in range(L)]
        arout = [[dp.tile([H, CH], BF16, name=f"ar_{l}_{n}", addr_space="Shared")
                  for n in range(NCH)] for l in range(L)]

        # ---------------- embedding: gather H-shard, transpose, AllGather/chunk
        def emit_embed_chunk(n, ep):
            for si in range(MB):
                st = MB * n + si
                gt = ep.tile([128, c["EMB"]], F32, tag="emg", bufs=2)
                nc.gpsimd.indirect_dma_start(
                    out=gt[:], out_offset=None, in_=embed_hs,
                    in_offset=bass.IndirectOffsetOnAxis(
                        ap=tok_sb[:, st:st + 1], axis=0),
                )
                for hb in range(c["EB"]):
                    tp = pp.tile([128, 128], F32, tag="mm", bufs=6, space="PSUM")
                    nc.tensor.transpose(tp[:], gt[:, hb * 128:(hb + 1) * 128],
                                        ident_f[:])
                    ts_ = ep.tile([128, 128], BF16, tag="emt", bufs=2)
                    nc.vector.tensor_copy(ts_[:], tp[:])
                    nc.sync.dma_start(
                        h0s_c[n][hb * 128:(hb + 1) * 128,
                                 si * 128:(si + 1) * 128], ts_[:])
            nc.gpsimd.collective_compute(
                "AllGather", OP.bypass, replica_groups=rg,
                ins=[h0s_c[n][:]], outs=[h0T_c[n][:]])

        # ---------------- per-chunk layernorm -> xhat tiles (bf16 in SBUF)
        def ln_chunk(n, h_src, ar_src, h_dst):
            """h_src/ar_src/h_dst: fn k -> DRAM ap [128, CH] (or None)."""
            sac = gp.tile([128, CH], BF16, tag="sac", bufs=1, name="sac")
            qac = gp.tile([128, CH], BF16, tag="qac", bufs=1, name="qac")
            for k in range(HT):
                hk = gp.tile([128, CH], BF16, tag="hh", bufs=3, name="hk")
                nc.sync.dma_start(hk[:], h_src(k))
                if ar_src is not None:
                    ak = gp.tile([128, CH], BF16, tag="ha", bufs=2, name="ak")
                    nc.sync.dma_start(ak[:], ar_src(k))
                    hs = gp.tile([128, CH], BF16, tag="hs", bufs=2, name="hs")
                    nc.vector.tensor_add(hs[:], hk[:], ak[:])
                    nc.sync.dma_start(h_dst(k), hs[:])
                    src_t = hs
                else:
                    src_t = hk
                sqk = gp.tile([128, CH], BF16, tag="sq", bufs=2, name="sqk")
                nc.scalar.activation(sqk[:], src_t[:], AF.Square)
                if k == 0:
                    nc.vector.tensor_copy(sac[:], src_t[:])
                    nc.vector.tensor_copy(qac[:], sqk[:])
                else:
                    nc.vector.tensor_add(sac[:], sac[:], src_t[:])
                    nc.vector.tensor_add(qac[:], qac[:], sqk[:])
            sar = gp.tile([128, CH], BF16, tag="sar", bufs=1, name="sar")
            qar = gp.tile([128, CH], BF16, tag="qar", bufs=1, name="qar")
            nc.gpsimd.partition_all_reduce(sar[:], sac[:], channels=128,
                                           reduce_op=bass_isa.ReduceOp.add)
            nc.gpsimd.partition_all_reduce(qar[:], qac[:], channels=128,
                                           reduce_op=bass_isa.ReduceOp.add)
            sA = gp.tile([1, CH], F32, tag="sA", bufs=1, name="sA")
            sB = gp.tile([1, CH], F32, tag="sB", bufs=1, name="sB")
            sAb = gp.tile([1, CH], BF16, tag="sAb", bufs=1, name="sAb")
            sBb = gp.tile([1, CH], BF16, tag="sBb", bufs=1, name="sBb")
            nc.vector.tensor_scalar_mul(sA[:], sar[0:1, :], 1.0 / H)
            nc.vector.tensor_scalar_mul(sB[:], qar[0:1, :], 1.0 / H)
            nc.vector.tensor_mul(sAb[:], sA[:], sA[:])            # mean^2
            nc.vector.tensor_tensor(out=sB[:], in0=sB[:], in1=sAb[:],
                                    op=OP.subtract)               # var
            nc.scalar.activation(sBb[:], sB[:], AF.Sqrt, bias=EPS)
            nc.vector.reciprocal(sB[:], sBb[:])                   # rstd (f32)
            nc.vector.tensor_mul(sA[:], sA[:], sB[:])             # mean*rstd
            nc.vector.tensor_scalar_mul(sAb[:], sA[:], -1.0)      # negmr
            nc.vector.tensor_copy(sBb[:], sB[:])
            rb = gp.tile([128, CH], BF16, tag="rb", bufs=1, name="rb")
            nb = gp.tile([128, CH], BF16, tag="nb", bufs=1, name="nb")
            nc.gpsimd.partition_broadcast(rb[:], sBb[0:1, :])
            nc.gpsimd.partition_broadcast(nb[:], sAb[0:1, :])
            h2_src = h_dst if (ar_src is not None) else h_src
            xh = []
            for k in range(HT):
                h2k = gp.tile([128, CH], BF16, tag="hh", bufs=3, name="h2k")
                nc.sync.dma_start(h2k[:], h2_src(k))
                xk = gp.tile([128, CH], BF16, tag=f"xh{k}", bufs=2, name="xk")
                nc.vector.tensor_mul(xk[:], h2k[:], rb[:])
                nc.vector.tensor_tensor(out=xk[:], in0=xk[:], in1=nb[:],
                                        op=OP.add)
                xh.append(xk)
            return xh

        def rope(x_ap, cs_t, sn_t):
            tmp = gp.tile([32, CH], BF16, tag="ro1", bufs=1, name="ro_tmp")
            ta = gp.tile([32, CH], BF16, tag="ro2", bufs=1, name="ro_ta")
            nc.sync.dma_start(tmp[0:16, :], x_ap[16:32, :])
            nc.sync.dma_start(tmp[16:32, :], x_ap[0:16, :])
            nc.vector.tensor_mul(ta[:], x_ap[0:32, :], cs_t[:])
            nc.vector.tensor_mul(tmp[:], tmp[:], sn_t[:])
            nc.vector.tensor_tensor(out=x_ap[0:32, :], in0=ta[:], in1=tmp[:],
                                    op=OP.add)

        # ---------------- one layer: front/back halves per chunk
        def make_layer(l, wp, lp):
            sl_l = l
            k_t = [lp.tile([128, S], BF16, name=f"k{j}_{l}") for j in range(NHL)]
            v_sb = [lp.tile([128, VCOL], BF16, name=f"v{st}_{l}")
                    for st in range(c["ST"])]
            wqk = [wp.tile([128, c["QKR"]], BF16, name=f"wqk{k}_{l}")
                   for k in range(HT)]
            wv = [wp.tile([128, VCOL], BF16, name=f"wv{k}_{l}") for k in range(HT)]
            wf1 = [wp.tile([128, FFL], BF16, name=f"wf1{k}_{l}") for k in range(HT)]
            wo = [wp.tile([128, H], BF16, name=f"wo{j}_{l}")
                  for j in range(VCOL // 128)]
            wf2 = [wp.tile([128, H], BF16, name=f"wf2{k}_{l}") for k in range(FMT)]
            for k in range(HT):
                nc.sync.dma_start(wqk[k][:], w_qkT[sl_l, k * 128:(k + 1) * 128, :])
            for k in range(HT):
                nc.sync.dma_start(wv[k][:], w_vT[sl_l, k * 128:(k + 1) * 128, :])
            for k in range(HT):
                nc.sync.dma_start(wf1[k][:], w_f1T[sl_l, k * 128:(k + 1) * 128, :])

            def load_back():
                for j in range(VCOL // 128):
                    nc.sync.dma_start(wo[j][:],
                                      w_oT[sl_l, j * 128:(j + 1) * 128, :])
                for k in range(FMT):
                    nc.sync.dma_start(wf2[k][:],
                                      w_f2T[sl_l, k * 128:(k + 1) * 128, :])

            q_cur = {}     # per chunk: q tiles per head
            g_cur = {}     # per chunk: g tiles

            def F(n):
                if l == 0:
                    xh = ln_chunk(n, lambda k: h0T_c[n][k * 128:(k + 1) * 128, :],
                                  None, None)
                else:
                    xh = ln_chunk(n, lambda k: h0T_c[n][k * 128:(k + 1) * 128, :],
                                  lambda k: arout[0][n][k * 128:(k + 1) * 128, :],
                                  lambda k: hb1_c[n][k * 128:(k + 1) * 128, :])
                # qkv
                qk_ps = [pp.tile([128, CH], F32, tag="mm", bufs=6, space="PSUM",
                                 name=f"qkps{b}") for b in range(c["NBLK"])]
                for k in range(HT):
                    for b in range(c["NBLK"]):
                        nc.tensor.matmul(
                            qk_ps[b][:], wqk[k][:, b * 128:(b + 1) * 128],
                            xh[k][:], start=(k == 0), stop=(k == HT - 1))
                cs_t = gp.tile([32, CH], BF16, tag="cs", bufs=1, name="cs_t")
                sn_t = gp.tile([32, CH], BF16, tag="sn", bufs=1, name="sn_t")
                nc.sync.dma_start(cs_t[:], cos32[:, n * CH:(n + 1) * CH])
                nc.sync.dma_start(sn_t[:], sin32[:, n * CH:(n + 1) * CH])
                qs = []
                for j in range(NHL):
                    qj = gp.tile([128, CH], BF16, tag=f"q{j}", bufs=2,
                                 name=f"q{j}")
                    nc.vector.tensor_scalar_add(qj[:], qk_ps[2 * j][:],
                                                bqk_sb[l][:, 2 * j:2 * j + 1])
                    ksl = k_t[j][:, n * CH:(n + 1) * CH]
                    nc.vector.tensor_scalar_add(ksl, qk_ps[2 * j + 1][:],
                                                bqk_sb[l][:, 2 * j + 1:2 * j + 2])
                    rope(qj[:], cs_t, sn_t)
                    rope(ksl, cs_t, sn_t)
                    qs.append(qj)
                q_cur[n] = qs
                # v (natural layout)
                for si in range(MB):
                    st = MB * n + si
                    vp = pp.tile([128, VCOL], F32, tag="mm", bufs=6,
                                 space="PSUM", name="vp")
                    for k in range(HT):
                        nc.tensor.matmul(
                            vp[:], xh[k][:, si * 128:(si + 1) * 128], wv[k][:],
                            start=(k == 0), stop=(k == HT - 1))
                    nc.vector.tensor_copy(v_sb[st][:], vp[:])
                # fc1 + gelu
                gs = []
                for f in range(FMT):
                    fp = pp.tile([128, CH], F32, tag="mm", bufs=6, space="PSUM",
                                 name="fp")
                    for k in range(HT):
                        nc.tensor.matmul(
                            fp[:], wf1[k][:, f * 128:(f + 1) * 128], xh[k][:],
                            start=(k == 0), stop=(k == HT - 1))
                    xb = gp.tile([128, CH], BF16, tag="gx", bufs=2, name="xb")
                    nc.vector.tensor_scalar_add(xb[:], fp[:],
                                                bf1_sb[l][:, f:f + 1])
                    x2 = gp.tile([128, CH], BF16, tag="g2", bufs=2, name="x2")
                    nc.vector.tensor_mul(x2[:], xb[:], xb[:])
                    nc.vector.tensor_scalar(
                        out=x2[:], in0=x2[:], scalar1=0.044715, scalar2=1.0,
                        op0=OP.mult, op1=OP.add)
                    nc.vector.tensor_mul(x2[:], x2[:], xb[:])
                    th = gp.tile([128, CH], BF16, tag="g3", bufs=2, name="th")
                    nc.scalar.activation(th[:], x2[:], AF.Tanh,
                                         scale=0.79788456)
                    nc.vector.tensor_scalar(
                        out=th[:], in0=th[:], scalar1=0.5, scalar2=0.5,
                        op0=OP.mult, op1=OP.add)
                    gf = gp.tile([128, CH], BF16, tag=f"g{f}", bufs=2,
                                 name=f"g{f}")
                    nc.vector.tensor_mul(gf[:], th[:], xb[:])
                    gs.append(gf)
                g_cur[n] = gs

            def B(n):
                # attention for q-chunk n over kv blocks 0..4n+3
                kbmax = MB * n + MB
                cxT = []
                for j in range(NHL):
                    cps = pp.tile([128, CH], F32, tag="ctx", bufs=2,
                                  space="PSUM", name="cps")
                    qj = q_cur[n][j]
                    esum = gp.tile([128, CH], BF16, tag="es", bufs=1,
                                   name="esum")

                    def emit_cd(kb, e_t, j=j, cps=cps, kbmax=kbmax):
                        nc.tensor.matmul(
                            cps[:], v_sb[kb][:, j * 128:(j + 1) * 128], e_t[:],
                            start=(kb == 0), stop=(kb == kbmax - 1))

                    pending = None
                    for kb in range(kbmax):
                        sp = pp.tile([128, CH], F32, tag="mm", bufs=6,
                                     space="PSUM", name="sp")
                        nc.tensor.matmul(sp[:],
                                         k_t[j][:, kb * 128:(kb + 1) * 128],
                                         qj[:], start=True, stop=True)
                        e_t = gp.tile([128, CH], BF16, tag="e", bufs=3,
                                      name="e_t")
                        nc.scalar.activation(e_t[:], sp[:], AF.Exp)
                        d = kb - MB * n
                        if d >= 0:
                            if d > 0:
                                nc.gpsimd.memset(e_t[:, 0:d * 128], 0.0)
                            nc.vector.tensor_mul(
                                e_t[:, d * 128:(d + 1) * 128],
                                e_t[:, d * 128:(d + 1) * 128], tri_sb[:])
                        if kb == 0:
                            nc.vector.tensor_copy(esum[:], e_t[:])
                        else:
                            nc.vector.tensor_add(esum[:], esum[:], e_t[:])
                        if pending is not None:
                            emit_cd(*pending)
                        pending = (kb, e_t)
                    emit_cd(*pending)
                    esr = gp.tile([128, CH], BF16, tag="db", bufs=2, name="esr")
                    nc.gpsimd.partition_all_reduce(esr[:], esum[:], channels=128,
                                                   reduce_op=bass_isa.ReduceOp.add)
                    db = gp.tile([128, CH], BF16, tag="db", bufs=2, name="db")
                    with nc.allow_low_precision(reason="softmax denom bf16"):
                        nc.vector.reciprocal(db[:], esr[:])
                    ct = gp.tile([128, CH], BF16, tag=f"cxT{j}", bufs=2,
                                 name=f"ct{j}")
                    nc.vector.tensor_mul(ct[:], cps[:], db[:])
                    cxT.append(ct)
                # attn_out + fc2 shared accumulation -> partial -> AllReduce
                gs = g_cur.pop(n)
                q_cur.pop(n)
                for m in range(HT):
                    op_ = pp.tile([128, CH], F32, tag="mm", bufs=6,
                                  space="PSUM", name="op_")
                    for j in range(VCOL // 128):
                        nc.tensor.matmul(op_[:], wo[j][:, m * 128:(m + 1) * 128],
                                         cxT[j][:], start=(j == 0), stop=False)
                    for kf in range(FMT):
                        nc.tensor.matmul(op_[:],
                                         wf2[kf][:, m * 128:(m + 1) * 128],
                                         gs[kf][:], start=False,
                                         stop=(kf == FMT - 1))
                    pot = gp.tile([128, CH], BF16, tag="po", bufs=2, name="pot")
                    nc.vector.tensor_scalar_add(pot[:], op_[:],
                                                bout_sb[l][:, m:m + 1])
                    nc.sync.dma_start(part[l][n][m * 128:(m + 1) * 128, :],
                                      pot[:])
                nc.gpsimd.collective_compute(
                    "AllReduce", OP.add, replica_groups=rg,
                    ins=[part[l][n][:]], outs=[arout[l][n][:]])

            return F, B, load_back

        # fix layer-1 h source: reads h0T + ar0 -> hb1
        # (ln_chunk call sites above use h0T_c for l==1 via closure)

        # ---------------- emission: embed, layers (F/B interleave), logits
        with tc.tile_pool(name="emb", bufs=1) as ep:
            for n in range(NCH):
                emit_embed_chunk(n, ep)

        for l in range(L):
            with ExitStack() as ls:
                wp = ls.enter_context(tc.tile_pool(name=f"w{l}", bufs=1))
                lp = ls.enter_context(tc.tile_pool(name=f"lp{l}", bufs=1))
                F, Bk, load_back = make_layer(l, wp, lp)
                F(0)
                F(1)
                load_back()
                Bk(0)
                F(2)
                Bk(1)
                F(3)
                Bk(2)
                Bk(3)

        # final LN + logits (vocab-sharded)
        with ExitStack() as fs:
            lgp = fs.enter_context(tc.tile_pool(name="lg", bufs=1))
            def final_ln(n):
                return ln_chunk(n,
                                lambda k: hb1_c[n][k * 128:(k + 1) * 128, :],
                                lambda k: arout[1][n][k * 128:(k + 1) * 128, :],
                                lambda k: hb2_c[n][k * 128:(k + 1) * 128, :])

            xhf_q = {0: final_ln(0)}

            def final_logits(n):
                xhf = xhf_q.pop(n)
                vn = c["VN"]
                for v in range(c["VNT"]):
                    wlt = []
                    for k in range(HT):
                        wk = lgp.tile([128, vn], BF16, tag="wl", bufs=24,
                                      name="wk")
                        nc.sync.dma_start(
                            wk[:], w_lgT[k * 128:(k + 1) * 128,
                                         v * vn:(v + 1) * vn])
                        wlt.append(wk)
                    for m in range(MB):
                        lp_ = pp.tile([128, vn], F32, tag="mm", bufs=6,
                                      space="PSUM", name="lp_")
                        for k in range(HT):
                            nc.tensor.matmul(
                                lp_[:], xhf[k][:, m * 128:(m + 1) * 128],
                                wlt[k][:], start=(k == 0), stop=(k == HT - 1))
                        lot = lgp.tile([128, vn], F32, tag="lo", bufs=2,
                                      name="lot")
                        nc.vector.tensor_copy(lot[:], lp_[:])
                        mg = MB * n + m
                        nc.sync.dma_start(
                            logits[mg * 128:(mg + 1) * 128,
                                   v * vn:(v + 1) * vn], lot[:])

            for n in range(NCH):
                if n + 1 < NCH:
                    xhf_q[n + 1] = final_ln(n + 1)
                final_logits(n)

    nc.compile()
    return nc


# ---------------------------------------------------------------- host prep ---


def prep_inputs(inputs, cfg):
    """Shard + preprocess full inputs -> list of per-core input maps."""
    c = derive(cfg)
    S, H, L, NH = c["S"], c["H"], c["L"], cfg["NH"]
    HD, ROT = c["HD"], c["ROT"]
    f32 = np.float32

    tokens = np.asarray(inputs["tokens"], np.int32)[0]          # [S]
    embed = np.asarray(inputs["embed"], f32)                    # [V, H]
    qkv_w = np.asarray(inputs["qkv_w"], f32)
    qkv_b = np.asarray(inputs["qkv_b"], f32)
    ow = np.asarray(inputs["attn_out_w"], f32)
    ob = np.asarray(inputs["attn_out_b"], f32)
    f1w = np.asarray(inputs["fc1_w"], f32)
    f1b = np.asarray(inputs["fc1_b"], f32)
    f2w = np.asarray(inputs["fc2_w"], f32)
    f2b = np.asarray(inputs["fc2_b"], f32)
    ln1_g = np.asarray(inputs["ln1_g"], f32)
    ln1_b = np.asarray(inputs["ln1_b"], f32)
    ln2_g = np.asarray(inputs["ln2_g"], f32)
    ln2_b = np.asarray(inputs["ln2_b"], f32)
    lnf_g = np.asarray(inputs["lnf_g"], f32)
    lnf_b = np.asarray(inputs["lnf_b"], f32)
    logits_w = np.asarray(inputs["logits_w"], f32)

    inv = 1.0 / (BASE ** (np.arange(0, ROT, 2, dtype=f32) / ROT))
    t = np.arange(S, dtype=f32)
    fr = np.outer(t, inv)                                       # [S, 16]
    cos16 = np.cos(fr).T.astype(f32)
    sin16 = np.sin(fr).T.astype(f32)
    cos32 = np.ascontiguousarray(np.vstack([cos16, cos16])).astype(BFNP)
    sin32 = np.ascontiguousarray(np.vstack([-sin16, sin16])).astype(BFNP)
    kk, qq = np.meshgrid(np.arange(128), np.arange(128), indexing="ij")
    tri = (qq >= kk).astype(BFNP)                               # [k, q]

    maps = []
    b_log_all = []
    for r in range(NC):
        m = {}
        m["tokens_t"] = np.ascontiguousarray(tokens.reshape(c["ST"], 128).T)
        ecols = slice(r * c["EMB"], (r + 1) * c["EMB"])
        m["embed_hs"] = np.ascontiguousarray(embed[:, ecols])
        m["cos32"], m["sin32"], m["tri"] = cos32, sin32, tri

        w_qkT = np.empty((L, H, c["QKR"]), BFNP)
        w_vT = np.empty((L, H, c["VCOL"]), BFNP)
        w_oT = np.empty((L, c["VCOL"], H), BFNP)
        w_f1T = np.empty((L, H, c["FFL"]), BFNP)
        w_f2T = np.empty((L, c["FFL"], H), BFNP)
        bqk = np.empty((L, 128, c["NBLK"]), f32)
        bf1 = np.empty((L, 128, c["FMT"]), f32)
        bout = np.empty((L, 128, c["HT"]), f32)
        heads = range(r * c["NHL"], (r + 1) * c["NHL"])
        for l in range(L):
            qk_rows, qk_bias = [], []
            v_rows, v_bias = [], []
            for h in heads:
                base = h * 3 * HD
                Wq = qkv_w[l, base:base + HD] * ln1_g[l][None, :]
                bq = qkv_b[l, base:base + HD] + qkv_w[l, base:base + HD] @ ln1_b[l]
                Wk = qkv_w[l, base + HD:base + 2 * HD] * ln1_g[l][None, :]
                bk = (qkv_b[l, base + HD:base + 2 * HD]
                      + qkv_w[l, base + HD:base + 2 * HD] @ ln1_b[l])
                Wv = qkv_w[l, base + 2 * HD:base + 3 * HD] * ln1_g[l][None, :]
                bv = (qkv_b[l, base + 2 * HD:base + 3 * HD]
                      + qkv_w[l, base + 2 * HD:base + 3 * HD] @ ln1_b[l])
                sc = 1.0 / math.sqrt(HD)
                qk_rows += [Wq * sc, Wk]
                qk_bias += [bq * sc, bk]
                v_rows.append(Wv)
                v_bias.append(bv)
            Wqk = np.concatenate(qk_rows, 0)                    # [QKR, H]
            w_qkT[l] = Wqk.T.astype(BFNP)
            bqk[l] = np.concatenate(qk_bias).reshape(c["NBLK"], 128).T
            Wv = np.concatenate(v_rows, 0)                      # [VCOL, H]
            w_vT[l] = Wv.T.astype(BFNP)
            bv_all = np.concatenate(v_bias)                     # [VCOL]
            ocols = slice(r * c["VCOL"], (r + 1) * c["VCOL"])
            Wo = ow[l][:, ocols]                                # [H, VCOL]
            w_oT[l] = Wo.T.astype(BFNP)
            frows = slice(r * c["FFL"], (r + 1) * c["FFL"])
            W1 = f1w[l][frows] * ln2_g[l][None, :]
            w_f1T[l] = W1.T.astype(BFNP)
            bf1[l] = (f1b[l][frows] + f1w[l][frows] @ ln2_b[l]).reshape(
                c["FMT"], 128).T
            fcols = slice(r * c["FFL"], (r + 1) * c["FFL"])
            w_f2T[l] = f2w[l][:, fcols].T.astype(BFNP)
            bo = (ob[l] + f2b[l]) / NC + Wo @ bv_all
            bout[l] = bo.reshape(c["HT"], 128).T
        m["w_qkT"], m["w_vT"], m["w_oT"] = w_qkT, w_vT, w_oT
        m["w_f1T"], m["w_f2T"] = w_f1T, w_f2T
        m["b_qk"], m["b_f1"], m["b_out"] = bqk, bf1, bout
        vrows = slice(r * c["VL"], (r + 1) * c["VL"])
        Wl = logits_w[vrows] * lnf_g[None, :]
        m["w_lgT"] = np.ascontiguousarray(Wl.T).astype(BFNP)
        b_log_all.append(logits_w[vrows] @ lnf_b)
        maps.append(m)
    return maps, b_log_all


# ---------------------------------------------------------------- entry ---

_PROGRAM_CACHE = {}


def _get_program(cfg_key):
    if cfg_key not in _PROGRAM_CACHE:
        _PROGRAM_CACHE[cfg_key] = build_program(REAL_CFG)
    return _PROGRAM_CACHE[cfg_key]


def _run(inputs, trace=False, cfg=None, nc=None):
    cfg = cfg or REAL_CFG
    c = derive(cfg)
    if nc is None:
        nc = _get_program("real")
    maps, b_log = prep_inputs(inputs, cfg)
    res = run_bass_kernel_spmd(nc, maps, list(range(NC)), trace=trace)
    shards = [res.results[r]["logits"] + b_log[r][None, :] for r in range(NC)]
    out = np.concatenate(shards, axis=1)[None].astype(np.float32)
    return out, res


def kernel(**inputs):
    out, _ = _run(inputs)
    return out
